# revision 36
# baseline (speedup 1.0000x reference)
"""BottleneckAttention TRN2 kernel: 8 NeuronCores, one (batch, head) pair per core.

Decomposition (per core, batch b / head i):
  q = (scale * Wq_i) @ x_b          [64, 4096]   (d-major)
  k = Wk_i @ x_b                    [64, 4096]
  vT = (Wv_i @ x_b)^T               [4096, 64]   (n-major, built chunkwise)
  Height rel-bias folded into the score matmul via an augmented contraction:
     K_aug = [k; Ih]  Q_aug = [q; RH^T]     (Ih[h',j] = 1 if j//64 == h')
     S^T[j,q] = K_aug^T Q_aug = content + height-bias
  Width rel-bias applied multiplicatively after exp (separability of exp):
     E = exp(S^T) * ew_dup[jw(j), q],  ew = exp(RW^T)
  PV + row-sums fused: vT_aug = [vT | 1] so out rows 0..63 = unnormalized
  attention output (transposed [d, q]), row 64 = softmax denominators.
  Output projection partial: P = Wout[:, i*64:(i+1)*64] @ out, then columns
  scaled by 1/sums (normalization commutes with the d-contraction).
Host sums the 4 per-head partials per batch and adds the residual x.

All inputs are pre-converted to bf16 on host (no on-device casts). The
steady-state pipeline is ACT(exp)-bound at ~1.1us per 128-key chunk; the
schedule keeps ACT saturated: PE builds/projections go to a dedicated PSUM
slot, psum->sbuf copies go to DVE, softmax denominators use the fast
approximate reciprocal, ew is exp'd in per-quarter chunks so quarter 0 can
start before the full width table is ready.
"""

import numpy as np
import ml_dtypes

import concourse.bass as bass
import concourse.bacc as bacc
import concourse.tile as tile
from concourse import mybir
from concourse.bass_utils import run_bass_kernel_spmd

F32 = mybir.dt.float32
BF16 = mybir.dt.bfloat16
AF = mybir.ActivationFunctionType

HEADS, B, C, HH, WW = 4, 2, 256, 64, 64
N = HH * WW           # 4096
DH = C // HEADS       # 64
NQ = 4                # query blocks
QB = N // NQ          # 1024 query cols per block
NJC = 32              # key chunks of 128
PVLAG = 6

# Schraudolph fast exp: exp(x) ~= bitcast_f32(int32(A*x + B)), ~4% max err.
# A handful of key chunks per quarter take this DVE path to offload the
# ACT engine (the pipeline bottleneck); the rest use exact ACT exp.
FEXP_A = 12102203.161561485     # 2^23 / ln 2
FEXP_B = float((127 << 23) - 486411)
FAST_JC = ()                    # DVE fast-exp: off (the TS+TT pair in the DVE
                                # FIFO delays e-muls -> PV stalls; net loss)
GPS_JC = ()                     # GpSimd e-muls: off (its SBUF port is shared
                                # with VectorE; measured 2.9us/mul, poisons DVE)
# e5m2 Schraudolph d-pairs: DVE computes e = bitcast_e5m2(uint8_sat(S' + RWB8))
# (S' = A8*logits via prescaled K_aug/rh rows; rw-bias folded additively into
# RWB8; -6 shift matches the ACT path's exp bias). PV for d-pairs runs as one
# fp8 DoubleRow matmul pair (2x contraction).
A8 = 4.0 / np.log(2.0)
SHIFT = float(np.log(8.0))      # folded into B8; compensated by vt8 *= 8
B8 = 60.0 - 0.25 - A8 * SHIFT
A16 = 128.0 / np.log(2.0)
B16 = 16256.0 - 7.42
D_CHUNKS = (0, 1)               # d-pair first: TTs hide behind prev-quarter ACT tail
HD_CHUNKS = (4, 5, 6, 7)        # half-d pairs: ACT exps cols 0:512, DVE TTs cols 512:1024
DRM = mybir.MatmulPerfMode.DoubleRow
U8 = mybir.dt.uint8
FP8E4 = mybir.dt.float8e4
FP8E5 = mybir.dt.float8e5


def _body(tc, io):
    from contextlib import ExitStack
    with ExitStack() as ctx:
        _body_inner(tc, io, ctx)


def _body_inner(tc, io, ctx):
    nc = tc.nc
    xb, wq, wk, wv, wo, relw, relh, ih, out, sums = (
        io["xb"], io["wq"], io["wk"], io["wv"], io["wo"],
        io["relw"], io["relh"], io["ih"], io["out"], io["sums"],
    )

    big = ctx.enter_context(tc.tile_pool(name="big", bufs=1))
    rot = ctx.enter_context(tc.tile_pool(name="rot", bufs=16))
    fpool = ctx.enter_context(tc.tile_pool(name="fpool", bufs=3))
    ep = ctx.enter_context(tc.tile_pool(name="ep", bufs=2))
    ed_pool = ctx.enter_context(tc.tile_pool(name="ed", bufs=2))
    spool = ctx.enter_context(tc.tile_pool(name="spool", bufs=2, space="PSUM"))
    opool = ctx.enter_context(tc.tile_pool(name="opool", bufs=1, space="PSUM"))
    ipool = ctx.enter_context(tc.tile_pool(name="ipool", bufs=1, space="PSUM"))
    dpool = ctx.enter_context(tc.tile_pool(name="dpool", bufs=2, space="DRAM"))

    # ---- SBUF tiles -------------------------------------------------
    xb_bf = big.tile([128, 2, N], BF16)
    wq_bf = big.tile([128, 2, DH], BF16)
    wk_bf = big.tile([128, 2, DH], BF16)
    wv_bf = big.tile([128, 2, DH], BF16)
    wo_bf = big.tile([64, 256], BF16)
    relw_bf = big.tile([64, 127], BF16)
    relh_bf = big.tile([64, 127], BF16)
    K_aug = big.tile([128, N], BF16)
    Q_aug = big.tile([128, N], BF16)
    ew_dup = big.tile([128, N], BF16)
    rwt = big.tile([64, N], BF16)
    vt_aug = big.tile([128, NJC, 65], BF16)
    vt8 = big.tile([128, NJC, 80], FP8E4)
    K_s8 = big.tile([128, N], BF16)
    RWB8 = big.tile([128, N], BF16)
    bias_m6 = big.tile([128, 1], F32)
    h_sb = big.tile([65, N], BF16)

    # ---- input DMAs ------------------------------------------------
    # First-needed data first; weight loads go on the gpsimd queue so the
    # sync queue's issue cost doesn't delay xb quarter 0.
    # xb pieces fan out over four queue engines (all idle this early) so the
    # transfers overlap instead of arriving ~2.3us apart off two queues.
    # One DMA per xb quarter (each fans out across all 16 DMA engines on its
    # own), spread over the three DMA-capable issue queues so all four are
    # in flight almost immediately.
    xv = xb.rearrange("(cc p) n -> p cc n", p=128)
    nc.gpsimd.dma_start(out=wq_bf, in_=wq.rearrange("(cc p) d -> p cc d", p=128))
    nc.sync.dma_start(out=xb_bf[:, :, 0:QB], in_=xv[:, :, 0:QB])
    nc.scalar.dma_start(out=xb_bf[:, :, bass.ts(1, QB)], in_=xv[:, :, bass.ts(1, QB)])
    for t_bf, t_d in ((wk_bf, wk), (wv_bf, wv)):
        nc.gpsimd.dma_start(out=t_bf, in_=t_d.rearrange("(cc p) d -> p cc d", p=128))
    nc.sync.dma_start(out=xb_bf[:, :, bass.ts(2, QB)], in_=xv[:, :, bass.ts(2, QB)])
    nc.scalar.dma_start(out=xb_bf[:, :, bass.ts(3, QB)], in_=xv[:, :, bass.ts(3, QB)])
    nc.sync.dma_start(out=relh_bf, in_=relh)
    nc.scalar.dma_start(out=relw_bf, in_=relw)
    # Ih rows of K_aug straight from dram (bf16, exact 0/1)
    nc.sync.dma_start(out=K_aug[64:128, :], in_=ih)
    nc.gpsimd.dma_start(out=wo_bf, in_=wo)

    nc.gpsimd.memset(vt_aug[:, :, 64:65], 1.0)
    nc.gpsimd.memset(vt8[:, :, 64:80], 0.0)
    nc.gpsimd.memset(vt8[:, :, 64:65], 8.0)
    nc.gpsimd.memset(bias_m6, -SHIFT)
    ones_row = big.tile([1, 128], BF16)
    nc.gpsimd.memset(ones_row, 1.0)

    # PE warm-up while the first xb quarter lands.
    warm = big.tile([128, 512], BF16)
    nc.vector.memset(warm, 0.0)
    for _ in range(11):
        wps = spool.tile([128, 512], F32, tag="sp")
        nc.tensor.matmul(wps, warm[:, 0:128], warm, start=True, stop=True)

    # ---- build helpers ---------------------------------------------
    # psum->sbuf copies: ACT while it is idle (prologue), DVE in-loop.
    def _copy(eng, dst, src, scl=None):
        if eng == "act":
            nc.scalar.activation(out=dst, in_=src, func=AF.Copy,
                                 scale=(1.0 if scl is None else scl))
        elif scl is not None:
            nc.vector.tensor_scalar(out=dst, in0=src, scalar1=scl, scalar2=None,
                                    op0=mybir.AluOpType.mult)
        else:
            nc.vector.tensor_copy(out=dst, in_=src)

    def qk_build(dst, w_bf, qq, pool, tag, eng="dve", scl=None):
        ps = pool.tile([128, QB], F32, tag=tag)
        for cc in range(2):
            for h in range(2):
                nc.tensor.matmul(
                    ps[0:64, bass.ts(h, 512)],
                    w_bf[:, cc, :],
                    xb_bf[:, cc, qq * QB + h * 512: qq * QB + (h + 1) * 512],
                    start=(cc == 0), stop=(cc == 1),
                )
        _copy(eng, dst[0:64, bass.ts(qq, QB)], ps[0:64, :], scl=scl)

    def rh_build(g, pool, tag, eng="dve"):
        # RH^T[jh, n=(x,y)] = sum_d relh[jh - x + 63, d] * q[d, n]
        ps = pool.tile([128, QB], F32, tag=tag)
        for xi in range(16):
            xx = g * 16 + xi
            nc.tensor.matmul(
                ps[0:64, bass.ts(xi, 64)],
                relh_bf[:, 63 - xx: 127 - xx],
                Q_aug[0:64, xx * 64: (xx + 1) * 64],
                start=True, stop=True,
            )
        _copy(eng, Q_aug[64:128, bass.ts(g, QB)], ps[0:64, :])

    q_xy = Q_aug[0:64, :].rearrange("d (x y) -> d x y", y=64)
    rwt_xy = rwt.rearrange("jw (x y) -> jw x y", y=64)

    def rw_build(g, pool, tag, eng="dve"):
        # RW^T[jw, n=(x,y)] = sum_d relw[jw - y + 63, d] * q[d, n]
        ps = pool.tile([128, QB], F32, tag=tag)
        for yi in range(16):
            yy = g * 16 + yi
            nc.tensor.matmul(
                ps[0:64, bass.ts(yi, 64)],
                relw_bf[:, 63 - yy: 127 - yy],
                q_xy[:, :, yy],
                start=True, stop=True,
            )
        # ps free layout is [yi, x]; rwt quarter slice wants [x, y].
        _copy(eng, rwt_xy[:, :, g * 16:(g + 1) * 16],
              ps[0:64, :].rearrange("p (yi x) -> p x yi", x=64))

    def vt_build(g, pool, tag, eng="dve"):
        ps = pool.tile([128, 8, 64], F32, tag=tag)
        for ci in range(8):
            chunk = g * 8 + ci
            for cc in range(2):
                nc.tensor.matmul(
                    ps[:, ci, :],
                    xb_bf[:, cc, chunk * 128: (chunk + 1) * 128],
                    wv_bf[:, cc, :],
                    start=(cc == 0), stop=(cc == 1),
                )
        _copy(eng, vt_aug[:, g * 8: (g + 1) * 8, 0:64], ps)
        nc.scalar.activation(out=vt8[:, g * 8:(g + 1) * 8, 0:64], in_=ps,
                             func=AF.Copy, scale=8.0)

    def ew_build(c):
        # ew chunk c covers query x-block c; bf16 Schraudolph on DVE (4x mode)
        nc.vector.tensor_scalar(
            out=ew_dup[0:64, bass.ts(c, QB)].bitcast(mybir.dt.int16),
            in0=rwt[:, bass.ts(c, QB)],
            scalar1=float(A16), scalar2=float(B16),
            op0=mybir.AluOpType.mult, op1=mybir.AluOpType.add)
        nc.vector.tensor_copy(out=ew_dup[64:128, bass.ts(c, QB)],
                              in_=ew_dup[0:64, bass.ts(c, QB)])

    # ---- prologue: all Q/K/V/rw builds -----------------------------
    # The main loop's quarter 0 has zero PE slack (S + deferred PV fill it),
    # so every build lives here, where ACT/DVE are otherwise idle. Builds
    # round-robin through 4 psum slots (spool x2 + ipool + opool, all free
    # before the loop) so a build never waits on the previous build's copy;
    # copies alternate between ACT and DVE so neither chain lags.
    slots = [(spool, "sp"), (spool, "sp"), (ipool, "ij"), (opool, "ov")]
    engs = ["act", "dve"]
    # rw right after the Q builds (it gates ew0 -> first e-mul) and ew0
    # emitted immediately after rw, BEFORE the K/V builds: engine FIFOs run
    # in emission order, so a late ew0 would queue behind the K/V copies on
    # ACT and delay every e-mul. Copy engines are assigned so the ACT chain
    # ahead of exp(0) is just {q0,q2,rw0,rw2,ew0,k0,v0}.
    builds = [("q", 0, "act"), ("q", 1, "dve"), ("rh", 0, "dve"),
              ("q", 2, "act"), ("q", 3, "dve"),
              ("rw", 0, "act"), ("rw", 1, "dve"), ("rw", 2, "act"),
              ("rw", 3, "dve"),
              ("ew", 0, None),
              ("k", 0, "act"), ("v", 0, "act")]
    nslot = 0
    for kind, idx, eng in builds:
        if kind == "ew":
            ew_build(0)
            continue
        pool, tag = slots[nslot % 4]
        nslot += 1
        if kind == "q":
            qk_build(Q_aug, wq_bf, idx, pool, tag, eng)
        elif kind == "k":
            qk_build(K_aug, wk_bf, idx, pool, tag, eng)
        elif kind == "v":
            vt_build(idx, pool, tag, eng)
        elif kind == "rh":
            rh_build(idx, pool, tag, eng)
        else:
            rw_build(idx, pool, tag, eng)

    # K_s8 = A8*K_aug for the d-pair's key columns only (chunks 0-1 live in
    # k-build 0's quarter, which IS prologue-built; k1-3 land in-loop later).
    assert max(D_CHUNKS) < 8
    nc.vector.tensor_scalar(out=K_s8[:, 0:QB], in0=K_aug[:, 0:QB],
                            scalar1=float(A8), scalar2=None,
                            op0=mybir.AluOpType.mult)
    # RWB8 = A8*rw + B8 (bf16, both partition halves) - feeds the d-pair TTs
    nc.vector.tensor_scalar(out=RWB8[0:64, :], in0=rwt, scalar1=float(A8),
                            scalar2=float(B8), op0=mybir.AluOpType.mult,
                            op1=mybir.AluOpType.add)
    nc.vector.tensor_scalar(out=RWB8[64:128, :], in0=rwt, scalar1=float(A8),
                            scalar2=float(B8), op0=mybir.AluOpType.mult,
                            op1=mybir.AluOpType.add)

    # ---- main attention loop ---------------------------------------
    # Per chunk: S^T matmul (PE) -> exp (ACT) -> *ew (DVE) -> PV (PE).
    # ACT is the bottleneck engine; everything else is scheduled around it.
    def proj(qqp, oh, pool=ipool, tag="ij"):
        pp = pool.tile([128, QB], F32, tag=tag)
        for h in range(2):
            nc.tensor.matmul(
                pp[:, bass.ts(h, 512)],
                wo_bf[:, oh * 128: (oh + 1) * 128],
                h_sb[0:64, qqp * QB + h * 512: qqp * QB + (h + 1) * 512],
                start=True, stop=True)
        osb = ep.tile([128, QB], BF16, tag="osb")
        nc.vector.tensor_copy(out=osb, in_=pp)
        eng = nc.sync if oh == 0 else nc.gpsimd
        eng.dma_start(
            out=out[oh * 128: (oh + 1) * 128, qqp * QB: (qqp + 1) * QB],
            in_=osb,
        )

    o_ps = None

    def drain(qqp):
        # rows 0:64 = unnormalized head-out, row 64 = softmax sums (host divides)
        nc.vector.tensor_copy(out=h_sb[:, bass.ts(qqp, QB)], in_=o_ps[0:65, :])
        nc.sync.dma_start(out=sums[:, qqp * QB:(qqp + 1) * QB],
                          in_=h_sb[64:65, bass.ts(qqp, QB)])

    for qq in range(NQ):
        if qq > 0:
            drain(qq - 1)
        o_ps = opool.tile([128, QB], F32, tag="ov")
        e_tiles = [None] * NJC

        e0_tiles = [None] * NJC
        hd_ed = {}

        def mul_stage(jc):
            eng = nc.gpsimd if jc in GPS_JC else nc.vector
            e = rot.tile([128, QB], BF16, tag="e")
            eng.tensor_mul(e, e0_tiles[jc], ew_dup[:, bass.ts(qq, QB)])
            e0_tiles[jc] = None
            e_tiles[jc] = e

        def s_stage(jc, do_mul=True):
            ps = spool.tile([128, QB], F32, tag="sp")
            if jc in HD_CHUNKS:
                ksrc_h = (K_aug, K_s8)
            else:
                ksrc_h = (K_s8, K_s8) if jc in D_CHUNKS else (K_aug, K_aug)
            for h in range(2):
                nc.tensor.matmul(
                    ps[:, bass.ts(h, 512)],
                    ksrc_h[h][:, jc * 128: (jc + 1) * 128],
                    Q_aug[:, qq * QB + h * 512: qq * QB + (h + 1) * 512],
                    start=True, stop=True,
                )
            if jc in HD_CHUNKS:
                # cols 0:512 exact exp (ACT) + mul; cols 512:1024 e5m2 TT (DVE)
                e0h = rot.tile([128, 512], BF16, tag="e0h")
                nc.scalar.activation(out=e0h, in_=ps[:, 0:512], func=AF.Exp)
                if jc % 2 == 0:
                    hd_ed[jc // 2] = ed_pool.tile([128, 2, 512], FP8E5,
                                                  tag="edh", name="edh_t")
                nc.vector.tensor_tensor(
                    out=hd_ed[jc // 2][:, jc % 2, :].bitcast(U8),
                    in0=ps[:, 512:1024],
                    in1=RWB8[:, qq * QB + 512: qq * QB + 1024],
                    op=mybir.AluOpType.add)
                eh = rot.tile([128, 512], BF16, tag="eh")
                nc.vector.tensor_mul(eh, e0h, ew_dup[:, qq * QB: qq * QB + 512])
                e_tiles[jc] = eh
            elif jc in D_CHUNKS:
                if jc % 2 == 0:
                    e_tiles[jc] = ed_pool.tile([128, 2, QB], FP8E5, tag="ed",
                                               name="ed_t")
                else:
                    e_tiles[jc] = e_tiles[jc - 1]
                nc.vector.tensor_tensor(
                    out=e_tiles[jc][:, jc % 2, :].bitcast(U8), in0=ps,
                    in1=RWB8[:, bass.ts(qq, QB)], op=mybir.AluOpType.add)
            else:
                e0 = rot.tile([128, QB], BF16, tag="e0")
                nc.scalar.activation(out=e0, in_=ps, func=AF.Exp)
                e0_tiles[jc] = e0
                if do_mul:
                    mul_stage(jc)

        def pv_stage(jc):
            if jc in HD_CHUNKS:
                nc.tensor.matmul(
                    o_ps[0:65, 0:512],
                    vt_aug[:, jc, :],
                    e_tiles[jc],
                    start=False, stop=False, skip_group_check=True)
                e_tiles[jc] = None
                if jc % 2 == 1:
                    pr = jc // 2
                    nc.tensor.matmul(
                        o_ps[0:80, 512:1024],
                        vt8[:, 2 * pr: 2 * pr + 2, :],
                        hd_ed.pop(pr),
                        start=False, stop=False, perf_mode=DRM,
                        skip_group_check=True)
                return
            if jc in D_CHUNKS:
                if jc % 2 == 0:
                    return
                pr = jc // 2
                for h in range(2):
                    nc.tensor.matmul(
                        o_ps[0:80, bass.ts(h, 512)],
                        vt8[:, 2 * pr: 2 * pr + 2, :],
                        e_tiles[jc][:, :, bass.ts(h, 512)],
                        start=(jc <= 1), stop=False, perf_mode=DRM,
                        skip_group_check=True)
                e_tiles[jc] = None
                e_tiles[jc - 1] = None
                return
            for h in range(2):
                nc.tensor.matmul(
                    o_ps[0:65, bass.ts(h, 512)],
                    vt_aug[:, jc, :],
                    e_tiles[jc][:, bass.ts(h, 512)],
                    start=(jc == 0 and 0 not in D_CHUNKS),
                    stop=(jc == NJC - 1),
                    skip_group_check=True,
                )
            e_tiles[jc] = None

        for t in range(NJC + PVLAG):
            if t < NJC:
                s_stage(t)
            if qq > 0:
                if t == 14:
                    proj(qq - 1, 0)
                elif t == 18:
                    proj(qq - 1, 1)
            if qq == 0:
                if t == 2:
                    qk_build(K_aug, wk_bf, 1, ipool, "ij", "dve")
                elif t == 5:
                    vt_build(1, ipool, "ij", "dve")
                elif t == 8:
                    qk_build(K_aug, wk_bf, 2, ipool, "ij", "dve")
                elif t == 11:
                    vt_build(2, ipool, "ij", "dve")
                elif t == 15:
                    qk_build(K_aug, wk_bf, 3, ipool, "ij", "dve")
                elif t == 18:
                    vt_build(3, ipool, "ij", "dve")
            if qq < NQ - 1:
                if t == 4 and qq > 0:
                    rh_build(qq + 1, ipool, "ij")
                elif t == 22 and qq == 0:
                    rh_build(qq + 1, ipool, "ij")
                elif t == 24:
                    ew_build(qq + 1)
            if t >= PVLAG:
                pv_stage(t - PVLAG)

    # final quarter epilogue on free S-pool slots
    drain(NQ - 1)
    proj(NQ - 1, 0, pool=spool, tag="sp")
    proj(NQ - 1, 1, pool=spool, tag="sp")


_NC_CACHE = {}


def _build():
    if "nc" in _NC_CACHE:
        return _NC_CACHE["nc"]
    nc = bacc.Bacc("TRN2", target_bir_lowering=False, debug=False, num_devices=8)
    io = {
        "xb": nc.dram_tensor("xb", [C, N], BF16, kind="ExternalInput").ap(),
        "wq": nc.dram_tensor("wq", [C, DH], BF16, kind="ExternalInput").ap(),
        "wk": nc.dram_tensor("wk", [C, DH], BF16, kind="ExternalInput").ap(),
        "wv": nc.dram_tensor("wv", [C, DH], BF16, kind="ExternalInput").ap(),
        "wo": nc.dram_tensor("wo", [DH, C], BF16, kind="ExternalInput").ap(),
        "relw": nc.dram_tensor("relw", [DH, 127], BF16, kind="ExternalInput").ap(),
        "relh": nc.dram_tensor("relh", [DH, 127], BF16, kind="ExternalInput").ap(),
        "ih": nc.dram_tensor("ih", [64, N], BF16, kind="ExternalInput").ap(),
        "out": nc.dram_tensor("out", [C, N], BF16, kind="ExternalOutput").ap(),
        "sums": nc.dram_tensor("sums", [1, N], BF16, kind="ExternalOutput").ap(),
    }
    with tile.TileContext(nc) as tc:
        _body(tc, io)
    nc.compile()
    _NC_CACHE["nc"] = nc
    return nc


_last_in_maps = None


def kernel(x, w_qkv, w_out, rel_height, rel_width):
    global _last_in_maps
    bf16 = ml_dtypes.bfloat16
    x = np.ascontiguousarray(np.asarray(x, np.float32))
    w_qkv = np.asarray(w_qkv, np.float32)
    w_out = np.asarray(w_out, np.float32)
    rel_height = np.asarray(rel_height, np.float32)
    rel_width = np.asarray(rel_width, np.float32)

    scale = np.float32(DH ** -0.5)
    ih_const = np.ascontiguousarray(
        np.repeat(np.eye(64, dtype=np.float32), 64, axis=1).astype(bf16))
    relw_t = np.ascontiguousarray(rel_width.T.astype(bf16))
    relh_t = np.ascontiguousarray(rel_height.T.astype(bf16))

    xb_bf = [np.ascontiguousarray(x[b].reshape(C, N).astype(bf16)) for b in range(B)]

    in_maps = []
    for g in range(8):
        b, i = divmod(g, HEADS)
        sl = slice(i * DH, (i + 1) * DH)
        in_maps.append({
            "xb": xb_bf[b],
            "wq": np.ascontiguousarray((w_qkv[i * DH:(i + 1) * DH] * scale).T.astype(bf16)),
            "wk": np.ascontiguousarray(w_qkv[C + i * DH: C + (i + 1) * DH].T.astype(bf16)),
            "wv": np.ascontiguousarray(w_qkv[2 * C + i * DH: 2 * C + (i + 1) * DH].T.astype(bf16)),
            "wo": np.ascontiguousarray(w_out[:, sl].T.astype(bf16)),
            "relw": relw_t,
            "relh": relh_t,
            "ih": ih_const,
        })

    _last_in_maps = in_maps
    nc = _build()
    res = run_bass_kernel_spmd(nc, in_maps, core_ids=list(range(8)))
    outf = np.empty((B, C, N), np.float32)
    for b in range(B):
        acc = x[b].reshape(C, N).copy()
        for i in range(HEADS):
            r = res.results[4 * b + i]
            acc += (np.asarray(r["out"]).astype(np.float32)
                    / np.asarray(r["sums"]).astype(np.float32))
        outf[b] = acc
    return outf.reshape(B, C, HH, WW)



# revision 37
# speedup vs baseline: 1.0176x; 1.0176x over previous
"""BottleneckAttention TRN2 kernel: 8 NeuronCores, one (batch, head) pair per core.

Decomposition (per core, batch b / head i):
  q = (scale * Wq_i) @ x_b          [64, 4096]   (d-major)
  k = Wk_i @ x_b                    [64, 4096]
  vT = (Wv_i @ x_b)^T               [4096, 64]   (n-major, built chunkwise)
  Height rel-bias folded into the score matmul via an augmented contraction:
     K_aug = [k; Ih]  Q_aug = [q; RH^T]     (Ih[h',j] = 1 if j//64 == h')
     S^T[j,q] = K_aug^T Q_aug = content + height-bias
  Width rel-bias applied multiplicatively after exp (separability of exp):
     E = exp(S^T) * ew_dup[jw(j), q],  ew = exp(RW^T)
  PV + row-sums fused: vT_aug = [vT | 1] so out rows 0..63 = unnormalized
  attention output (transposed [d, q]), row 64 = softmax denominators.
  Output projection partial: P = Wout[:, i*64:(i+1)*64] @ out, then columns
  scaled by 1/sums (normalization commutes with the d-contraction).
Host sums the 4 per-head partials per batch and adds the residual x.

All inputs are pre-converted to bf16 on host (no on-device casts). The
steady-state pipeline is ACT(exp)-bound at ~1.1us per 128-key chunk; the
schedule keeps ACT saturated: PE builds/projections go to a dedicated PSUM
slot, psum->sbuf copies go to DVE, softmax denominators use the fast
approximate reciprocal, ew is exp'd in per-quarter chunks so quarter 0 can
start before the full width table is ready.
"""

import numpy as np
import ml_dtypes

import concourse.bass as bass
import concourse.bacc as bacc
import concourse.tile as tile
from concourse import mybir
from concourse.bass_utils import run_bass_kernel_spmd

F32 = mybir.dt.float32
BF16 = mybir.dt.bfloat16
AF = mybir.ActivationFunctionType

HEADS, B, C, HH, WW = 4, 2, 256, 64, 64
N = HH * WW           # 4096
DH = C // HEADS       # 64
NQ = 4                # query blocks
QB = N // NQ          # 1024 query cols per block
NJC = 32              # key chunks of 128
PVLAG = 6

# Schraudolph fast exp: exp(x) ~= bitcast_f32(int32(A*x + B)), ~4% max err.
# A handful of key chunks per quarter take this DVE path to offload the
# ACT engine (the pipeline bottleneck); the rest use exact ACT exp.
FEXP_A = 12102203.161561485     # 2^23 / ln 2
FEXP_B = float((127 << 23) - 486411)
FAST_JC = ()                    # DVE fast-exp: off (the TS+TT pair in the DVE
                                # FIFO delays e-muls -> PV stalls; net loss)
GPS_JC = ()                     # GpSimd e-muls: off (its SBUF port is shared
                                # with VectorE; measured 2.9us/mul, poisons DVE)
# e5m2 Schraudolph d-pairs: DVE computes e = bitcast_e5m2(uint8_sat(S' + RWB8))
# (S' = A8*logits via prescaled K_aug/rh rows; rw-bias folded additively into
# RWB8; -6 shift matches the ACT path's exp bias). PV for d-pairs runs as one
# fp8 DoubleRow matmul pair (2x contraction).
A8 = 4.0 / np.log(2.0)
SHIFT = float(np.log(8.0))      # folded into B8; compensated by vt8 *= 8
B8 = 60.0 - 0.25 - A8 * SHIFT
A16 = 128.0 / np.log(2.0)
B16 = 16256.0 - 7.42
D_CHUNKS = (0, 1)               # d-pair first: TTs hide behind prev-quarter ACT tail
HD_CHUNKS = (4, 5)              # half-d pair: ACT exps cols 0:512, DVE TTs cols 512:1024
DRM = mybir.MatmulPerfMode.DoubleRow
U8 = mybir.dt.uint8
FP8E4 = mybir.dt.float8e4
FP8E5 = mybir.dt.float8e5


def _body(tc, io):
    from contextlib import ExitStack
    with ExitStack() as ctx:
        _body_inner(tc, io, ctx)


def _body_inner(tc, io, ctx):
    nc = tc.nc
    xb, wq, wk, wv, wo, relw, relh, ih, out, sums = (
        io["xb"], io["wq"], io["wk"], io["wv"], io["wo"],
        io["relw"], io["relh"], io["ih"], io["out"], io["sums"],
    )

    big = ctx.enter_context(tc.tile_pool(name="big", bufs=1))
    rot = ctx.enter_context(tc.tile_pool(name="rot", bufs=16))
    fpool = ctx.enter_context(tc.tile_pool(name="fpool", bufs=3))
    ep = ctx.enter_context(tc.tile_pool(name="ep", bufs=2))
    ed_pool = ctx.enter_context(tc.tile_pool(name="ed", bufs=2))
    spool = ctx.enter_context(tc.tile_pool(name="spool", bufs=2, space="PSUM"))
    opool = ctx.enter_context(tc.tile_pool(name="opool", bufs=1, space="PSUM"))
    ipool = ctx.enter_context(tc.tile_pool(name="ipool", bufs=1, space="PSUM"))
    dpool = ctx.enter_context(tc.tile_pool(name="dpool", bufs=2, space="DRAM"))

    # ---- SBUF tiles -------------------------------------------------
    xb_bf = big.tile([128, 2, N], BF16)
    wq_bf = big.tile([128, 2, DH], BF16)
    wk_bf = big.tile([128, 2, DH], BF16)
    wv_bf = big.tile([128, 2, DH], BF16)
    wo_bf = big.tile([64, 256], BF16)
    relw_bf = big.tile([64, 127], BF16)
    relh_bf = big.tile([64, 127], BF16)
    K_aug = big.tile([128, N], BF16)
    Q_aug = big.tile([128, N], BF16)
    ew_dup = big.tile([128, N], BF16)
    rwt = big.tile([64, N], BF16)
    vt_aug = big.tile([128, NJC, 65], BF16)
    vt8 = big.tile([128, NJC, 80], FP8E4)
    K_s8 = big.tile([128, N], BF16)
    RWB8 = big.tile([128, N], BF16)
    bias_m6 = big.tile([128, 1], F32)
    h_sb = big.tile([65, N], BF16)

    # ---- input DMAs ------------------------------------------------
    # First-needed data first; weight loads go on the gpsimd queue so the
    # sync queue's issue cost doesn't delay xb quarter 0.
    # xb pieces fan out over four queue engines (all idle this early) so the
    # transfers overlap instead of arriving ~2.3us apart off two queues.
    # One DMA per xb quarter (each fans out across all 16 DMA engines on its
    # own), spread over the three DMA-capable issue queues so all four are
    # in flight almost immediately.
    xv = xb.rearrange("(cc p) n -> p cc n", p=128)
    nc.gpsimd.dma_start(out=wq_bf, in_=wq.rearrange("(cc p) d -> p cc d", p=128))
    nc.sync.dma_start(out=xb_bf[:, :, 0:QB], in_=xv[:, :, 0:QB])
    nc.scalar.dma_start(out=xb_bf[:, :, bass.ts(1, QB)], in_=xv[:, :, bass.ts(1, QB)])
    for t_bf, t_d in ((wk_bf, wk), (wv_bf, wv)):
        nc.gpsimd.dma_start(out=t_bf, in_=t_d.rearrange("(cc p) d -> p cc d", p=128))
    nc.sync.dma_start(out=xb_bf[:, :, bass.ts(2, QB)], in_=xv[:, :, bass.ts(2, QB)])
    nc.scalar.dma_start(out=xb_bf[:, :, bass.ts(3, QB)], in_=xv[:, :, bass.ts(3, QB)])
    nc.sync.dma_start(out=relh_bf, in_=relh)
    nc.scalar.dma_start(out=relw_bf, in_=relw)
    # Ih rows of K_aug straight from dram (bf16, exact 0/1)
    nc.sync.dma_start(out=K_aug[64:128, :], in_=ih)
    nc.gpsimd.dma_start(out=wo_bf, in_=wo)

    nc.gpsimd.memset(vt_aug[:, :, 64:65], 1.0)
    nc.gpsimd.memset(vt8[:, :, 64:80], 0.0)
    nc.gpsimd.memset(vt8[:, :, 64:65], 8.0)
    nc.gpsimd.memset(bias_m6, -SHIFT)
    ones_row = big.tile([1, 128], BF16)
    nc.gpsimd.memset(ones_row, 1.0)

    # PE warm-up while the first xb quarter lands.
    warm = big.tile([128, 512], BF16)
    nc.vector.memset(warm, 0.0)
    for _ in range(11):
        wps = spool.tile([128, 512], F32, tag="sp")
        nc.tensor.matmul(wps, warm[:, 0:128], warm, start=True, stop=True)

    # ---- build helpers ---------------------------------------------
    # psum->sbuf copies: ACT while it is idle (prologue), DVE in-loop.
    def _copy(eng, dst, src, scl=None):
        if eng == "act":
            nc.scalar.activation(out=dst, in_=src, func=AF.Copy,
                                 scale=(1.0 if scl is None else scl))
        elif scl is not None:
            nc.vector.tensor_scalar(out=dst, in0=src, scalar1=scl, scalar2=None,
                                    op0=mybir.AluOpType.mult)
        else:
            nc.vector.tensor_copy(out=dst, in_=src)

    def qk_build(dst, w_bf, qq, pool, tag, eng="dve", scl=None):
        ps = pool.tile([128, QB], F32, tag=tag)
        for cc in range(2):
            for h in range(2):
                nc.tensor.matmul(
                    ps[0:64, bass.ts(h, 512)],
                    w_bf[:, cc, :],
                    xb_bf[:, cc, qq * QB + h * 512: qq * QB + (h + 1) * 512],
                    start=(cc == 0), stop=(cc == 1),
                )
        _copy(eng, dst[0:64, bass.ts(qq, QB)], ps[0:64, :], scl=scl)

    def rh_build(g, pool, tag, eng="dve"):
        # RH^T[jh, n=(x,y)] = sum_d relh[jh - x + 63, d] * q[d, n]
        ps = pool.tile([128, QB], F32, tag=tag)
        for xi in range(16):
            xx = g * 16 + xi
            nc.tensor.matmul(
                ps[0:64, bass.ts(xi, 64)],
                relh_bf[:, 63 - xx: 127 - xx],
                Q_aug[0:64, xx * 64: (xx + 1) * 64],
                start=True, stop=True,
            )
        _copy(eng, Q_aug[64:128, bass.ts(g, QB)], ps[0:64, :])

    q_xy = Q_aug[0:64, :].rearrange("d (x y) -> d x y", y=64)
    rwt_xy = rwt.rearrange("jw (x y) -> jw x y", y=64)

    def rw_build(g, pool, tag, eng="dve"):
        # RW^T[jw, n=(x,y)] = sum_d relw[jw - y + 63, d] * q[d, n]
        ps = pool.tile([128, QB], F32, tag=tag)
        for yi in range(16):
            yy = g * 16 + yi
            nc.tensor.matmul(
                ps[0:64, bass.ts(yi, 64)],
                relw_bf[:, 63 - yy: 127 - yy],
                q_xy[:, :, yy],
                start=True, stop=True,
            )
        # ps free layout is [yi, x]; rwt quarter slice wants [x, y].
        _copy(eng, rwt_xy[:, :, g * 16:(g + 1) * 16],
              ps[0:64, :].rearrange("p (yi x) -> p x yi", x=64))

    def vt_build(g, pool, tag, eng="dve"):
        ps = pool.tile([128, 8, 64], F32, tag=tag)
        for ci in range(8):
            chunk = g * 8 + ci
            for cc in range(2):
                nc.tensor.matmul(
                    ps[:, ci, :],
                    xb_bf[:, cc, chunk * 128: (chunk + 1) * 128],
                    wv_bf[:, cc, :],
                    start=(cc == 0), stop=(cc == 1),
                )
        _copy(eng, vt_aug[:, g * 8: (g + 1) * 8, 0:64], ps)
        nc.scalar.activation(out=vt8[:, g * 8:(g + 1) * 8, 0:64], in_=ps,
                             func=AF.Copy, scale=8.0)

    def ew_build(c):
        # ew chunk c covers query x-block c; bf16 Schraudolph on DVE (4x mode)
        nc.vector.tensor_scalar(
            out=ew_dup[0:64, bass.ts(c, QB)].bitcast(mybir.dt.int16),
            in0=rwt[:, bass.ts(c, QB)],
            scalar1=float(A16), scalar2=float(B16),
            op0=mybir.AluOpType.mult, op1=mybir.AluOpType.add)
        nc.vector.tensor_copy(out=ew_dup[64:128, bass.ts(c, QB)],
                              in_=ew_dup[0:64, bass.ts(c, QB)])

    # ---- prologue: all Q/K/V/rw builds -----------------------------
    # The main loop's quarter 0 has zero PE slack (S + deferred PV fill it),
    # so every build lives here, where ACT/DVE are otherwise idle. Builds
    # round-robin through 4 psum slots (spool x2 + ipool + opool, all free
    # before the loop) so a build never waits on the previous build's copy;
    # copies alternate between ACT and DVE so neither chain lags.
    slots = [(spool, "sp"), (spool, "sp"), (ipool, "ij"), (opool, "ov")]
    engs = ["act", "dve"]
    # rw right after the Q builds (it gates ew0 -> first e-mul) and ew0
    # emitted immediately after rw, BEFORE the K/V builds: engine FIFOs run
    # in emission order, so a late ew0 would queue behind the K/V copies on
    # ACT and delay every e-mul. Copy engines are assigned so the ACT chain
    # ahead of exp(0) is just {q0,q2,rw0,rw2,ew0,k0,v0}.
    builds = [("q", 0, "act"), ("q", 1, "dve"), ("rh", 0, "dve"),
              ("q", 2, "act"), ("q", 3, "dve"),
              ("rw", 0, "act"), ("rw", 1, "dve"), ("rw", 2, "act"),
              ("rw", 3, "dve"),
              ("ew", 0, None),
              ("k", 0, "act"), ("v", 0, "act")]
    nslot = 0
    for kind, idx, eng in builds:
        if kind == "ew":
            ew_build(0)
            continue
        pool, tag = slots[nslot % 4]
        nslot += 1
        if kind == "q":
            qk_build(Q_aug, wq_bf, idx, pool, tag, eng)
        elif kind == "k":
            qk_build(K_aug, wk_bf, idx, pool, tag, eng)
        elif kind == "v":
            vt_build(idx, pool, tag, eng)
        elif kind == "rh":
            rh_build(idx, pool, tag, eng)
        else:
            rw_build(idx, pool, tag, eng)

    # K_s8 = A8*K_aug for the d-pair's key columns only (chunks 0-1 live in
    # k-build 0's quarter, which IS prologue-built; k1-3 land in-loop later).
    assert max(D_CHUNKS) < 8
    nc.vector.tensor_scalar(out=K_s8[:, 0:QB], in0=K_aug[:, 0:QB],
                            scalar1=float(A8), scalar2=None,
                            op0=mybir.AluOpType.mult)
    # RWB8 = A8*rw + B8 (bf16, both partition halves) - feeds the d-pair TTs
    nc.vector.tensor_scalar(out=RWB8[0:64, :], in0=rwt, scalar1=float(A8),
                            scalar2=float(B8), op0=mybir.AluOpType.mult,
                            op1=mybir.AluOpType.add)
    nc.vector.tensor_scalar(out=RWB8[64:128, :], in0=rwt, scalar1=float(A8),
                            scalar2=float(B8), op0=mybir.AluOpType.mult,
                            op1=mybir.AluOpType.add)

    # ---- main attention loop ---------------------------------------
    # Per chunk: S^T matmul (PE) -> exp (ACT) -> *ew (DVE) -> PV (PE).
    # ACT is the bottleneck engine; everything else is scheduled around it.
    def proj(qqp, oh, pool=ipool, tag="ij"):
        pp = pool.tile([128, QB], F32, tag=tag)
        for h in range(2):
            nc.tensor.matmul(
                pp[:, bass.ts(h, 512)],
                wo_bf[:, oh * 128: (oh + 1) * 128],
                h_sb[0:64, qqp * QB + h * 512: qqp * QB + (h + 1) * 512],
                start=True, stop=True)
        osb = ep.tile([128, QB], BF16, tag="osb")
        nc.vector.tensor_copy(out=osb, in_=pp)
        eng = nc.sync if oh == 0 else nc.gpsimd
        eng.dma_start(
            out=out[oh * 128: (oh + 1) * 128, qqp * QB: (qqp + 1) * QB],
            in_=osb,
        )

    o_ps = None

    def drain(qqp):
        # rows 0:64 = unnormalized head-out, row 64 = softmax sums (host divides)
        nc.vector.tensor_copy(out=h_sb[:, bass.ts(qqp, QB)], in_=o_ps[0:65, :])
        nc.sync.dma_start(out=sums[:, qqp * QB:(qqp + 1) * QB],
                          in_=h_sb[64:65, bass.ts(qqp, QB)])

    for qq in range(NQ):
        if qq > 0:
            drain(qq - 1)
        o_ps = opool.tile([128, QB], F32, tag="ov")
        e_tiles = [None] * NJC

        e0_tiles = [None] * NJC
        hd_ed = [None]

        def mul_stage(jc):
            eng = nc.gpsimd if jc in GPS_JC else nc.vector
            e = rot.tile([128, QB], BF16, tag="e")
            eng.tensor_mul(e, e0_tiles[jc], ew_dup[:, bass.ts(qq, QB)])
            e0_tiles[jc] = None
            e_tiles[jc] = e

        def s_stage(jc, do_mul=True):
            ps = spool.tile([128, QB], F32, tag="sp")
            if jc in HD_CHUNKS:
                ksrc_h = (K_aug, K_s8)
            else:
                ksrc_h = (K_s8, K_s8) if jc in D_CHUNKS else (K_aug, K_aug)
            for h in range(2):
                nc.tensor.matmul(
                    ps[:, bass.ts(h, 512)],
                    ksrc_h[h][:, jc * 128: (jc + 1) * 128],
                    Q_aug[:, qq * QB + h * 512: qq * QB + (h + 1) * 512],
                    start=True, stop=True,
                )
            if jc in HD_CHUNKS:
                # cols 0:512 exact exp (ACT) + mul; cols 512:1024 e5m2 TT (DVE)
                e0h = rot.tile([128, 512], BF16, tag="e0h")
                nc.scalar.activation(out=e0h, in_=ps[:, 0:512], func=AF.Exp)
                if jc % 2 == 0:
                    hd_ed[0] = ed_pool.tile([128, 2, 512], FP8E5, tag="edh",
                                            name="edh_t")
                nc.vector.tensor_tensor(
                    out=hd_ed[0][:, jc % 2, :].bitcast(U8), in0=ps[:, 512:1024],
                    in1=RWB8[:, qq * QB + 512: qq * QB + 1024],
                    op=mybir.AluOpType.add)
                eh = rot.tile([128, 512], BF16, tag="eh")
                nc.vector.tensor_mul(eh, e0h, ew_dup[:, qq * QB: qq * QB + 512])
                e_tiles[jc] = eh
            elif jc in D_CHUNKS:
                if jc % 2 == 0:
                    e_tiles[jc] = ed_pool.tile([128, 2, QB], FP8E5, tag="ed",
                                               name="ed_t")
                else:
                    e_tiles[jc] = e_tiles[jc - 1]
                nc.vector.tensor_tensor(
                    out=e_tiles[jc][:, jc % 2, :].bitcast(U8), in0=ps,
                    in1=RWB8[:, bass.ts(qq, QB)], op=mybir.AluOpType.add)
            else:
                e0 = rot.tile([128, QB], BF16, tag="e0")
                nc.scalar.activation(out=e0, in_=ps, func=AF.Exp)
                e0_tiles[jc] = e0
                if do_mul:
                    mul_stage(jc)

        def pv_stage(jc):
            if jc in HD_CHUNKS:
                nc.tensor.matmul(
                    o_ps[0:65, 0:512],
                    vt_aug[:, jc, :],
                    e_tiles[jc],
                    start=False, stop=False, skip_group_check=True)
                e_tiles[jc] = None
                if jc % 2 == 1:
                    pr = jc // 2
                    nc.tensor.matmul(
                        o_ps[0:80, 512:1024],
                        vt8[:, 2 * pr: 2 * pr + 2, :],
                        hd_ed[0],
                        start=False, stop=False, perf_mode=DRM,
                        skip_group_check=True)
                    hd_ed[0] = None
                return
            if jc in D_CHUNKS:
                if jc % 2 == 0:
                    return
                pr = jc // 2
                for h in range(2):
                    nc.tensor.matmul(
                        o_ps[0:80, bass.ts(h, 512)],
                        vt8[:, 2 * pr: 2 * pr + 2, :],
                        e_tiles[jc][:, :, bass.ts(h, 512)],
                        start=(jc <= 1), stop=False, perf_mode=DRM,
                        skip_group_check=True)
                e_tiles[jc] = None
                e_tiles[jc - 1] = None
                return
            for h in range(2):
                nc.tensor.matmul(
                    o_ps[0:65, bass.ts(h, 512)],
                    vt_aug[:, jc, :],
                    e_tiles[jc][:, bass.ts(h, 512)],
                    start=(jc == 0 and 0 not in D_CHUNKS),
                    stop=(jc == NJC - 1),
                    skip_group_check=True,
                )
            e_tiles[jc] = None

        for t in range(NJC + PVLAG):
            if t < NJC:
                s_stage(t)
            if qq > 0:
                if t == 14:
                    proj(qq - 1, 0)
                elif t == 18:
                    proj(qq - 1, 1)
            if qq == 0:
                if t == 2:
                    qk_build(K_aug, wk_bf, 1, ipool, "ij", "dve")
                elif t == 5:
                    vt_build(1, ipool, "ij", "dve")
                elif t == 8:
                    qk_build(K_aug, wk_bf, 2, ipool, "ij", "dve")
                elif t == 11:
                    vt_build(2, ipool, "ij", "dve")
                elif t == 15:
                    qk_build(K_aug, wk_bf, 3, ipool, "ij", "dve")
                elif t == 18:
                    vt_build(3, ipool, "ij", "dve")
            if qq < NQ - 1:
                if t == 4 and qq > 0:
                    rh_build(qq + 1, ipool, "ij")
                elif t == 22 and qq == 0:
                    rh_build(qq + 1, ipool, "ij")
                elif t == 24:
                    ew_build(qq + 1)
            if t >= PVLAG:
                pv_stage(t - PVLAG)

    # final quarter epilogue on free S-pool slots
    drain(NQ - 1)
    proj(NQ - 1, 0, pool=spool, tag="sp")
    proj(NQ - 1, 1, pool=spool, tag="sp")


_NC_CACHE = {}


def _build():
    if "nc" in _NC_CACHE:
        return _NC_CACHE["nc"]
    nc = bacc.Bacc("TRN2", target_bir_lowering=False, debug=False, num_devices=8)
    io = {
        "xb": nc.dram_tensor("xb", [C, N], BF16, kind="ExternalInput").ap(),
        "wq": nc.dram_tensor("wq", [C, DH], BF16, kind="ExternalInput").ap(),
        "wk": nc.dram_tensor("wk", [C, DH], BF16, kind="ExternalInput").ap(),
        "wv": nc.dram_tensor("wv", [C, DH], BF16, kind="ExternalInput").ap(),
        "wo": nc.dram_tensor("wo", [DH, C], BF16, kind="ExternalInput").ap(),
        "relw": nc.dram_tensor("relw", [DH, 127], BF16, kind="ExternalInput").ap(),
        "relh": nc.dram_tensor("relh", [DH, 127], BF16, kind="ExternalInput").ap(),
        "ih": nc.dram_tensor("ih", [64, N], BF16, kind="ExternalInput").ap(),
        "out": nc.dram_tensor("out", [C, N], BF16, kind="ExternalOutput").ap(),
        "sums": nc.dram_tensor("sums", [1, N], BF16, kind="ExternalOutput").ap(),
    }
    with tile.TileContext(nc) as tc:
        _body(tc, io)
    nc.compile()
    _NC_CACHE["nc"] = nc
    return nc


_last_in_maps = None


def kernel(x, w_qkv, w_out, rel_height, rel_width):
    global _last_in_maps
    bf16 = ml_dtypes.bfloat16
    x = np.ascontiguousarray(np.asarray(x, np.float32))
    w_qkv = np.asarray(w_qkv, np.float32)
    w_out = np.asarray(w_out, np.float32)
    rel_height = np.asarray(rel_height, np.float32)
    rel_width = np.asarray(rel_width, np.float32)

    scale = np.float32(DH ** -0.5)
    ih_const = np.ascontiguousarray(
        np.repeat(np.eye(64, dtype=np.float32), 64, axis=1).astype(bf16))
    relw_t = np.ascontiguousarray(rel_width.T.astype(bf16))
    relh_t = np.ascontiguousarray(rel_height.T.astype(bf16))

    xb_bf = [np.ascontiguousarray(x[b].reshape(C, N).astype(bf16)) for b in range(B)]

    in_maps = []
    for g in range(8):
        b, i = divmod(g, HEADS)
        sl = slice(i * DH, (i + 1) * DH)
        in_maps.append({
            "xb": xb_bf[b],
            "wq": np.ascontiguousarray((w_qkv[i * DH:(i + 1) * DH] * scale).T.astype(bf16)),
            "wk": np.ascontiguousarray(w_qkv[C + i * DH: C + (i + 1) * DH].T.astype(bf16)),
            "wv": np.ascontiguousarray(w_qkv[2 * C + i * DH: 2 * C + (i + 1) * DH].T.astype(bf16)),
            "wo": np.ascontiguousarray(w_out[:, sl].T.astype(bf16)),
            "relw": relw_t,
            "relh": relh_t,
            "ih": ih_const,
        })

    _last_in_maps = in_maps
    nc = _build()
    res = run_bass_kernel_spmd(nc, in_maps, core_ids=list(range(8)))
    outf = np.empty((B, C, N), np.float32)
    for b in range(B):
        acc = x[b].reshape(C, N).copy()
        for i in range(HEADS):
            r = res.results[4 * b + i]
            acc += (np.asarray(r["out"]).astype(np.float32)
                    / np.asarray(r["sums"]).astype(np.float32))
        outf[b] = acc
    return outf.reshape(B, C, HH, WW)

